# revision 35
# baseline (speedup 1.0000x reference)
"""GQA kernel for Trainium2, sharded over 8 NeuronCores.

Problem: B=2, S=2048, D=2048, H=16 q-heads, HKV=4 kv-heads, DH=128.
Sharding: core = b*4 + g handles batch b and kv-head group g (4 q-heads).
Each core computes its group's Q/K/V projections, attention, and the
row-sharded slice of the output projection; the host sums the 4 partial
outputs per batch (Wo row-parallel reduction).

v3 layout strategy (mixed precision, PSUM accumulation always fp32):
  - Streams qT/kT/vT arrive TRANSPOSED [D, S] in bf16; weights bf16
    (wv fp16).  All matmuls run at 1 PE cycle/row (vs 4 for fp32).
  - qT streamed in 256-col chunks and wq in two half-DMAs so the first
    projection matmul issues ~7us after kernel start.
  - kp/qp: projected k/q kept transposed [DH, S] fp16 (dh on partitions).
  - vp: projected v in NATURAL layout [s, dh] fp16, computed directly
    with vT chunks as the stationary operand (no PE transposes).
  - scores^T = K_block @ Q^T per (kc pair, qblock) into a 2-bank psum
    tile; one exp activation per [128, 2, 512] tile (amortizes ACT's
    fixed ~370ns per-op overhead).
  - P^T tiles fp16; per-partition partial row sums via DVE adds across
    kc tiles; full softmax denominator via gpsimd partition_all_reduce
    (result replicated across partitions), then avn = av / rsum with a
    single DVE tensor-tensor divide.  No rowsum/broadcast matmuls.
  - out partial = (avn concat heads) @ Wo_g with avn^T slices stationary,
    wo moving bf16; psum -> fp32 staging -> DMA per 128-row block.
  - out-projection groups interleave into the NEXT q-block's attention
    iterations (borrowing the po psum ring) so the PE stays busy while
    ACT works through the exps.
"""

import math
import sys

import numpy as np
import ml_dtypes

if "/opt/trn_rl_repo" not in sys.path:
    sys.path.insert(0, "/opt/trn_rl_repo")

S = 2048
D = 2048
DH = 128
NH = 4  # q-heads per core (one GQA group)
DC = D // 128  # contraction chunks for projections
KC = S // 128  # k-chunks for attention
QB = 512  # q-block (matmul moving free dim)
NQB = S // QB
QCH = 256  # qT stream chunk width
SCALE = 1.0 / math.sqrt(DH)
N_CORES = 8

LAST_EXEC_NS = None
LAST_RESULTS = None

_PROGRAM = None


def _emit(tc, nc, mybir, bass_isa, qT, kT, vT, wq, wk, wv, wo, out):
    f32 = mybir.dt.float32
    f16 = mybir.dt.float16
    bf16 = mybir.dt.bfloat16
    Exp = mybir.ActivationFunctionType.Exp

    qT_r = qT[:].rearrange("(dc p) s -> p dc s", p=128)  # [128, DC, S] bf16
    kT_r = kT[:].rearrange("(dc p) s -> p dc s", p=128)
    vT_r = vT[:].rearrange("(dc p) s -> p dc s", p=128)
    wq_r = wq[:].rearrange("(dc p) c -> p dc c", p=128)  # [128, DC, 512]
    # wk/wv arrive pre-rearranged [128, DC*DH] (4KB contiguous per
    # partition) so their DMAs run at full descriptor efficiency.
    wk_r = wk[:].rearrange("p (dc c) -> p dc c", c=DH)  # [128, DC, 128]
    wv_r = wv[:].rearrange("p (dc c) -> p dc c", c=DH)
    wo_r = wo[:].rearrange("(ck p) d -> p ck d", p=128)  # [128, NH, D]
    out_r = out[:].rearrange("(sc p) d -> p sc d", p=128)  # [128, S//128, D]

    with tc.tile_pool(name="persist", bufs=1) as persist:
        kp = persist.tile([128, S], f16)  # k_proj^T
        vp = persist.tile([128, KC, DH], f16)  # v_proj natural, by kchunk
        qp = persist.tile([128, NH, S], f16)  # q_proj^T per local head
        avn = persist.tile([128, NH, S], f16)  # normalized attn out^T

        wq_sb = persist.tile([128, DC, NH * DH], bf16, tag="wq")
        wk_sb = persist.tile([128, DC, DH], bf16, tag="wk")
        wv_sb = persist.tile([128, DC, DH], f16, tag="wv")
        wo_sb = persist.tile([128, NH, D], bf16, tag="wo")

        xq_tiles = {}
        xs_tiles = {}
        xv_tiles = {}

        def vproj_unit(c, kl, psum_pool, tag):
            xt = xv_tiles[c]
            ps = psum_pool.tile([128, 512], f32, tag=tag, name=f"pv{c}_{kl}")
            for dc in range(DC):
                nc.tensor.matmul(
                    ps[:, 0:DH],
                    lhsT=xt[:, dc, kl * 128:(kl + 1) * 128],
                    rhs=wv_sb[:, dc, :],
                    start=(dc == 0),
                    stop=(dc == DC - 1),
                )
            nc.vector.tensor_copy(vp[:, c * (QB // 128) + kl, :], ps[:, 0:DH])

        # vT chunks (and the deferred q7 chunk) outlive the projection scope:
        # V2/V3 projection and Qproj(7) are interleaved into the first
        # attention block.
        with tc.tile_pool(name="xv", bufs=3) as xv_pool, \
             tc.tile_pool(name="xq7", bufs=1) as xq7_pool:

            def dma_v(c):
                xt = xv_pool.tile([128, DC, QB], bf16, tag="xv", name=f"xv{c}")
                nc.sync.dma_start(out=xt, in_=vT_r[:, :, c * QB:(c + 1) * QB])
                xv_tiles[c] = xt

            with tc.tile_pool(name="xq", bufs=3) as xq_pool, \
                 tc.tile_pool(name="xstream", bufs=2) as xs_pool, \
                 tc.tile_pool(name="proj_psum", bufs=2, space="PSUM") as pj_psum, \
                 tc.tile_pool(name="projv_psum", bufs=2, space="PSUM") as pv_psum:

                def dma_q(c):
                    if c == NQB * 2 - 1:
                        pool, tag = xq7_pool, "xq7"
                    else:
                        pool, tag = xq_pool, "xq"
                    xt = pool.tile([128, DC, QCH], bf16, tag=tag,
                                   name=f"xq{c}")
                    nc.sync.dma_start(out=xt, in_=qT_r[:, :, c * QCH:(c + 1) * QCH])
                    xq_tiles[c] = xt

                def dma_k(c):
                    xt = xs_pool.tile([128, DC, QB], bf16, tag="xs",
                                      name=f"xk{c}")
                    nc.sync.dma_start(out=xt, in_=kT_r[:, :, c * QB:(c + 1) * QB])
                    xs_tiles[c] = xt

                # DMA issue order == transfer order (serial DMA pool in the
                # sim): prioritize the q path so the PE starts ~4.5us in,
                # then trickle kT/vT behind while Qproj chews.  The first
                # chunk and wq are split so the very first half-contraction
                # can begin after only two ~1.5us transfers.
                xt0 = xq_pool.tile([128, DC, QCH], bf16, tag="xq", name="xq0")
                xq_tiles[0] = xt0
                nc.sync.dma_start(out=xt0[:, 0:4, :], in_=qT_r[:, 0:4, 0:QCH])
                nc.sync.dma_start(out=wq_sb[:, 0:4, 0:256], in_=wq_r[:, 0:4, 0:256])
                nc.sync.dma_start(out=xt0[:, 4:8, :], in_=qT_r[:, 4:8, 0:QCH])
                nc.sync.dma_start(out=wq_sb[:, 4:8, 0:256], in_=wq_r[:, 4:8, 0:256])
                nc.sync.dma_start(out=xt0[:, 8:16, :], in_=qT_r[:, 8:16, 0:QCH])
                nc.sync.dma_start(out=wq_sb[:, 8:16, 0:256], in_=wq_r[:, 8:16, 0:256])
                nc.sync.dma_start(out=wq_sb[:, :, 256:512], in_=wq_r[:, :, 256:512])
                dma_q(1)
                dma_q(2)
                nc.sync.dma_start(out=wk_sb, in_=wk_r)
                dma_k(0)
                dma_q(3)
                dma_k(1)
                dma_q(4)
                dma_q(5)
                dma_k(2)
                dma_q(6)
                dma_k(3)
                nc.sync.dma_start(out=wv_sb, in_=wv_r)
                dma_v(0)
                dma_v(1)
                dma_v(2)
                dma_v(3)
                dma_q(7)
                # wo is not needed until the first out-proj group (~t+120us);
                # issuing it last keeps vT ahead of the V projection.
                nc.sync.dma_start(out=wo_sb, in_=wo_r)

                def qproj_head(c, h, psum_pool, tag):
                    xt = xq_tiles[c]
                    ps = psum_pool.tile([128, QB], f32, tag=tag)
                    for dc in range(DC):
                        nc.tensor.matmul(
                            ps[:, 0:QCH],
                            lhsT=wq_sb[:, dc, h * DH:(h + 1) * DH],
                            rhs=xt[:, dc, :],
                            start=(dc == 0),
                            stop=(dc == DC - 1),
                        )
                    nc.vector.tensor_copy(
                        qp[:, h, c * QCH:(c + 1) * QCH], ps[:, 0:QCH])

                def qproj(c):
                    for h in range(NH):
                        qproj_head(c, h, pj_psum, "pj")

                def kproj(c):
                    xt = xs_tiles[c]
                    ps = pj_psum.tile([128, QB], f32, tag="pj")
                    for dc in range(DC):
                        nc.tensor.matmul(
                            ps, lhsT=wk_sb[:, dc, :], rhs=xt[:, dc, :],
                            start=(dc == 0), stop=(dc == DC - 1),
                        )
                    nc.vector.tensor_copy(kp[:, c * QB:(c + 1) * QB], ps)

                # PE emission order tuned against DMA arrival times.
                # V2/V3 and Qproj(7) are deferred into the first attention
                # block so the PE has work while the tail of the stream
                # arrives.
                qproj(0)
                qproj(1)
                qproj(2)
                kproj(0)
                qproj(3)
                qproj(4)
                kproj(1)
                qproj(5)
                qproj(6)
                kproj(2)
                kproj(3)
                for kl in range(4):
                    vproj_unit(0, kl, pv_psum, "pv")
                for kl in range(4):
                    vproj_unit(1, kl, pv_psum, "pv")

            # ---- attention + interleaved output projection ----
            with tc.tile_pool(name="s_psum", bufs=2, space="PSUM") as s_psum, \
                 tc.tile_pool(name="av_psum", bufs=2, space="PSUM") as av_psum, \
                 tc.tile_pool(name="po_psum", bufs=2, space="PSUM") as po_psum, \
                 tc.tile_pool(name="pt_pool", bufs=6) as pt_pool, \
                 tc.tile_pool(name="small", bufs=3) as small_pool, \
                 tc.tile_pool(name="ostage", bufs=3) as ostage:

                def o_groups(qb):
                    """Generator: emit output projection for q rows of block
                    qb in 16 resumable chunks.  Each [sc, db] psum group is
                    split: ck0-2 accumulate immediately, ck3 (which reads the
                    h3-gated avn slice) is deferred one chunk so the PE has
                    runnable matmuls while the last head's normalization
                    lands."""
                    def part1(sc, db, n):
                        if qb == NQB - 1 and n % 2 == 1:
                            # drain phase: the scores ring is idle — borrow
                            # its banks to double the pipeline depth
                            st = s_psum.tile([128, 2, QB], f32, tag="s",
                                             name=f"pos{sc}_{db}")
                            po = st[:, 0, :]
                        else:
                            po = po_psum.tile([128, 512], f32, tag="po",
                                              name=f"po{sc}_{db}")
                        for ck in range(NH - 1):
                            nc.tensor.matmul(
                                po,
                                lhsT=avn[:, ck, sc * 128:(sc + 1) * 128],
                                rhs=wo_sb[:, ck, db * 512:(db + 1) * 512],
                                start=(ck == 0), stop=False,
                            )
                        return po

                    def finish(po, ot, sc, db, n):
                        nc.tensor.matmul(
                            po,
                            lhsT=avn[:, NH - 1, sc * 128:(sc + 1) * 128],
                            rhs=wo_sb[:, NH - 1, db * 512:(db + 1) * 512],
                            start=False, stop=True,
                        )
                        dst = ot[:, db * 512:(db + 1) * 512]
                        if n % 2 == 1:
                            nc.scalar.copy(dst, po)
                        else:
                            nc.vector.tensor_copy(dst, po)
                        nc.sync.dma_start(
                            out=out_r[:, sc, db * 512:(db + 1) * 512],
                            in_=dst)

                    prev = None
                    n = 0
                    for sc in range(qb * NQB, (qb + 1) * NQB):
                        ot = ostage.tile([128, D], bf16, tag="ot",
                                         name=f"ot{sc}")
                        for db in range(NH):
                            po = part1(sc, db, n)
                            if prev is not None:
                                finish(*prev, n)
                            n += 1
                            prev = (po, ot, sc, db)
                            yield
                    finish(*prev, n)

                def v_units():
                    for c in (2, 3):
                        for kl in range(4):
                            vproj_unit(c, kl, po_psum, "po")
                            yield

                # deferred Qproj(7) head-groups fill the ACT-paced idle of
                # the first attention block's later head iterations
                q7_fills = {
                    (0, 1, 1): 0, (0, 1, 5): 1, (0, 2, 1): 2, (0, 3, 1): 3,
                }

                pending_o = None
                pending_v = v_units()
                for qb in range(NQB):
                    qs = slice(qb * QB, (qb + 1) * QB)
                    for h in range(NH):
                        av = av_psum.tile([128, QB], f32, tag="av")
                        ptsum = small_pool.tile([128, QB], f16, tag="ptsum")
                        for pair in range(KC // 2):
                            ss = s_psum.tile([128, 2, QB], f32, tag="s")
                            for j in range(2):
                                kc = pair * 2 + j
                                nc.tensor.matmul(
                                    ss[:, j, :],
                                    lhsT=kp[:, kc * 128:(kc + 1) * 128],
                                    rhs=qp[:, h, qs],
                                    start=True, stop=True,
                                )
                            pt = pt_pool.tile([128, 2, QB], f16, tag="pt")
                            nc.scalar.activation(pt, ss, Exp, scale=SCALE)
                            # V2/V3 projection units ride in the first
                            # attention iteration's PE stream, ahead of the
                            # AV matmuls that consume them (unit k is needed
                            # by the AV of pair 4+k//2; placed one pair
                            # before that to track the vT DMA arrival).
                            if pending_v is not None and qb == 0 and h == 0:
                                if pair >= 1:
                                    next(pending_v, None)
                                if pair == 7:
                                    next(pending_v, None)
                            for j in range(2):
                                kc = pair * 2 + j
                                nc.tensor.matmul(
                                    av, lhsT=vp[:, kc, :], rhs=pt[:, j, :],
                                    start=(kc == 0), stop=(kc == KC - 1),
                                )
                            # adds expressed as (a*1)+b: InstTensorScalarPtr
                            # runs in the 4x_2p DVE mode (TensorTensor only
                            # gets 2x), halving the row-sum chain cost
                            def stt_add(out, a, b):
                                nc.vector.scalar_tensor_tensor(
                                    out, a, 1.0, b,
                                    mybir.AluOpType.mult, mybir.AluOpType.add)
                            if pair == 0:
                                stt_add(ptsum, pt[:, 0, :], pt[:, 1, :])
                            else:
                                stt_add(ptsum, ptsum, pt[:, 0, :])
                                stt_add(ptsum, ptsum, pt[:, 1, :])
                            # interleave one out-proj group into the PE stream
                            if pending_o is not None and pair % 2 == 1:
                                next(pending_o, None)
                            q7h = q7_fills.get((qb, h, pair))
                            if q7h is not None:
                                qproj_head(NQB * 2 - 1, q7h, po_psum, "po")
                        # softmax denominator: partition all-reduce (result
                        # replicated across partitions) on the idle gpsimd,
                        # then normalize via reciprocal + multiply on DVE.
                        rsum = small_pool.tile([128, QB], f32, tag="rsum")
                        nc.gpsimd.partition_all_reduce(
                            rsum, ptsum, channels=128,
                            reduce_op=bass_isa.ReduceOp.add)
                        rinv = small_pool.tile([128, QB], f32, tag="rinv")
                        if qb == NQB - 1 and h == NH - 1:
                            # the drain's deferred ck3 matmuls consume avn in
                            # 128-col slices; normalize in quarters so the
                            # first out-proj groups unblock ~1us earlier
                            for i in range(4):
                                sl = slice(i * 128, (i + 1) * 128)
                                nc.vector.reciprocal(rinv[:, sl], rsum[:, sl])
                                nc.vector.tensor_mul(
                                    avn[:, h, qb * QB + i * 128:
                                        qb * QB + (i + 1) * 128],
                                    av[:, sl], rinv[:, sl])
                        else:
                            nc.vector.reciprocal(rinv, rsum)
                            nc.vector.tensor_mul(avn[:, h, qs], av, rinv)
                    # drain leftover groups of the previous block, then arm
                    # this block's out-projection for interleaving
                    if pending_o is not None:
                        for _ in pending_o:
                            pass
                    pending_o = o_groups(qb)
                for _ in pending_o:
                    pass


def build_program():
    global _PROGRAM
    if _PROGRAM is not None:
        return _PROGRAM
    import concourse.tile as tile
    from concourse import bacc, bass_isa, mybir

    f32 = mybir.dt.float32
    bf16 = mybir.dt.bfloat16
    f16 = mybir.dt.float16
    nc = bacc.Bacc("TRN2", target_bir_lowering=False, debug=False)
    qT = nc.declare_dram_parameter("qT", [D, S], bf16, isOutput=False)
    kT = nc.declare_dram_parameter("kT", [D, S], bf16, isOutput=False)
    vT = nc.declare_dram_parameter("vT", [D, S], bf16, isOutput=False)
    wq = nc.declare_dram_parameter("wq", [D, NH * DH], bf16, isOutput=False)
    # wk/wv pre-rearranged on host to [128, DC*DH] (partition-major)
    wk = nc.declare_dram_parameter("wk", [128, DC * DH], bf16, isOutput=False)
    wv = nc.declare_dram_parameter("wv", [128, DC * DH], f16, isOutput=False)
    wo = nc.declare_dram_parameter("wo", [NH * DH, D], bf16, isOutput=False)
    out = nc.declare_dram_parameter("out", [S, D], bf16, isOutput=True)

    with tile.TileContext(nc) as tc:
        _emit(tc, nc, mybir, bass_isa, qT, kT, vT, wq, wk, wv, wo, out)

    nc.finalize()
    _PROGRAM = nc
    return nc


def _pmajor(w):
    # [D, DH] -> [128, DC*DH]: row (dc*128+p) becomes partition p, block dc
    return np.ascontiguousarray(
        w.reshape(DC, 128, DH).transpose(1, 0, 2).reshape(128, DC * DH))


def make_in_maps(query, key, value, Wq, Wk, Wv, Wo):
    bff = ml_dtypes.bfloat16
    in_maps = []
    for core in range(N_CORES):
        b, g = core // 4, core % 4
        in_maps.append({
            "qT": np.ascontiguousarray(np.asarray(query[b], np.float32).T).astype(bff),
            "kT": np.ascontiguousarray(np.asarray(key[b], np.float32).T).astype(bff),
            "vT": np.ascontiguousarray(np.asarray(value[b], np.float32).T).astype(bff),
            "wq": np.asarray(Wq[:, g * 512:(g + 1) * 512], np.float32).astype(bff),
            "wk": _pmajor(np.asarray(Wk[:, g * 128:(g + 1) * 128], np.float32)).astype(bff),
            "wv": _pmajor(np.asarray(Wv[:, g * 128:(g + 1) * 128], np.float32)).astype(np.float16),
            "wo": np.asarray(Wo[g * 512:(g + 1) * 512, :], np.float32).astype(bff),
        })
    return in_maps


def kernel(query, key, value, mask, Wq, Wk, Wv, Wo):
    global LAST_EXEC_NS, LAST_RESULTS
    del mask  # all-ones in this problem; softmax masking is a no-op
    nc = build_program()
    in_maps = make_in_maps(query, key, value, Wq, Wk, Wv, Wo)

    from concourse.bass_utils import run_bass_kernel_spmd

    res = run_bass_kernel_spmd(nc, in_maps, core_ids=list(range(N_CORES)))
    LAST_EXEC_NS = res.exec_time_ns
    LAST_RESULTS = res
    outs = [np.asarray(r["out"], dtype=np.float32) for r in res.results]
    full = np.empty((2, S, D), np.float32)
    for b in range(2):
        full[b] = outs[b * 4] + outs[b * 4 + 1] + outs[b * 4 + 2] + outs[b * 4 + 3]
    return full


# revision 37
# speedup vs baseline: 1.0929x; 1.0929x over previous
"""GQA kernel for Trainium2, sharded over 8 NeuronCores.

Problem: B=2, S=2048, D=2048, H=16 q-heads, HKV=4 kv-heads, DH=128.
Sharding: core = b*4 + g handles batch b and kv-head group g (4 q-heads).
Each core computes its group's Q/K/V projections, attention, and the
row-sharded slice of the output projection; the host sums the 4 partial
outputs per batch (Wo row-parallel reduction).

v3 layout strategy (mixed precision, PSUM accumulation always fp32):
  - Streams qT/kT/vT arrive TRANSPOSED [D, S] in bf16; weights bf16
    (wv fp16).  All matmuls run at 1 PE cycle/row (vs 4 for fp32).
  - qT streamed in 256-col chunks and wq in two half-DMAs so the first
    projection matmul issues ~7us after kernel start.
  - kp/qp: projected k/q kept transposed [DH, S] fp16 (dh on partitions).
  - vp: projected v in NATURAL layout [s, dh] fp16, computed directly
    with vT chunks as the stationary operand (no PE transposes).
  - scores^T = K_block @ Q^T per (kc pair, qblock) into a 2-bank psum
    tile; one exp activation per [128, 2, 512] tile (amortizes ACT's
    fixed ~370ns per-op overhead).
  - P^T tiles fp16; per-partition partial row sums via DVE adds across
    kc tiles; full softmax denominator via gpsimd partition_all_reduce
    (result replicated across partitions), then avn = av / rsum with a
    single DVE tensor-tensor divide.  No rowsum/broadcast matmuls.
  - out partial = (avn concat heads) @ Wo_g with avn^T slices stationary,
    wo moving bf16; psum -> fp32 staging -> DMA per 128-row block.
  - out-projection groups interleave into the NEXT q-block's attention
    iterations (borrowing the po psum ring) so the PE stays busy while
    ACT works through the exps.
"""

import math
import sys

import numpy as np
import ml_dtypes

if "/opt/trn_rl_repo" not in sys.path:
    sys.path.insert(0, "/opt/trn_rl_repo")

S = 2048
D = 2048
DH = 128
NH = 4  # q-heads per core (one GQA group)
DC = D // 128  # contraction chunks for projections
KC = S // 128  # k-chunks for attention
QB = 512  # q-block (matmul moving free dim)
NQB = S // QB
QCH = 256  # qT stream chunk width
SCALE = 1.0 / math.sqrt(DH)
N_CORES = 8

LAST_EXEC_NS = None
LAST_RESULTS = None

_PROGRAM = None


def _emit(tc, nc, mybir, bass_isa, qT, kT, vT, wq, wk, wv, wo, out):
    f32 = mybir.dt.float32
    f16 = mybir.dt.float16
    bf16 = mybir.dt.bfloat16
    Exp = mybir.ActivationFunctionType.Exp

    qT_r = qT[:].rearrange("(dc p) s -> p dc s", p=128)  # [128, DC, S] bf16
    kT_r = kT[:].rearrange("(dc p) s -> p dc s", p=128)
    vT_r = vT[:].rearrange("(dc p) s -> p dc s", p=128)
    wq_r = wq[:].rearrange("(dc p) c -> p dc c", p=128)  # [128, DC, 512]
    # wk/wv arrive pre-rearranged [128, DC*DH] (4KB contiguous per
    # partition) so their DMAs run at full descriptor efficiency.
    wk_r = wk[:].rearrange("p (dc c) -> p dc c", c=DH)  # [128, DC, 128]
    wv_r = wv[:].rearrange("p (dc c) -> p dc c", c=DH)
    wo_r = wo[:].rearrange("(ck p) d -> p ck d", p=128)  # [128, NH, D]
    out_r = out[:].rearrange("(sc p) d -> p sc d", p=128)  # [128, S//128, D]

    with tc.tile_pool(name="persist", bufs=1) as persist:
        kp = persist.tile([128, S], f16)  # k_proj^T
        vp = persist.tile([128, KC, DH], f16)  # v_proj natural, by kchunk
        qp = persist.tile([128, NH, S], f16)  # q_proj^T per local head
        avn = persist.tile([128, NH, S], f16)  # normalized attn out^T

        wq_sb = persist.tile([128, DC, NH * DH], bf16, tag="wq")
        wk_sb = persist.tile([128, DC, DH], bf16, tag="wk")
        wv_sb = persist.tile([128, DC, DH], f16, tag="wv")
        wo_sb = persist.tile([128, NH, D], bf16, tag="wo")

        xq_tiles = {}
        xs_tiles = {}
        xv_tiles = {}

        def vproj_unit(c, kl, psum_pool, tag):
            xt = xv_tiles[c]
            ps = psum_pool.tile([128, 512], f32, tag=tag, name=f"pv{c}_{kl}")
            for dc in range(DC):
                nc.tensor.matmul(
                    ps[:, 0:DH],
                    lhsT=xt[:, dc, kl * 128:(kl + 1) * 128],
                    rhs=wv_sb[:, dc, :],
                    start=(dc == 0),
                    stop=(dc == DC - 1),
                )
            nc.vector.tensor_copy(vp[:, c * (QB // 128) + kl, :], ps[:, 0:DH])

        # vT chunks (and the deferred q7 chunk) outlive the projection scope:
        # V2/V3 projection and Qproj(7) are interleaved into the first
        # attention block.
        with tc.tile_pool(name="xv", bufs=3) as xv_pool, \
             tc.tile_pool(name="xq7", bufs=1) as xq7_pool:

            def dma_v(c):
                xt = xv_pool.tile([128, DC, QB], bf16, tag="xv", name=f"xv{c}")
                nc.sync.dma_start(out=xt, in_=vT_r[:, :, c * QB:(c + 1) * QB])
                xv_tiles[c] = xt

            with tc.tile_pool(name="xq", bufs=3) as xq_pool, \
                 tc.tile_pool(name="xstream", bufs=2) as xs_pool, \
                 tc.tile_pool(name="proj_psum", bufs=2, space="PSUM") as pj_psum, \
                 tc.tile_pool(name="projv_psum", bufs=2, space="PSUM") as pv_psum:

                def dma_q(c):
                    if c == NQB * 2 - 1:
                        pool, tag = xq7_pool, "xq7"
                    else:
                        pool, tag = xq_pool, "xq"
                    xt = pool.tile([128, DC, QCH], bf16, tag=tag,
                                   name=f"xq{c}")
                    nc.sync.dma_start(out=xt, in_=qT_r[:, :, c * QCH:(c + 1) * QCH])
                    xq_tiles[c] = xt

                def dma_k(c):
                    xt = xs_pool.tile([128, DC, QB], bf16, tag="xs",
                                      name=f"xk{c}")
                    nc.sync.dma_start(out=xt, in_=kT_r[:, :, c * QB:(c + 1) * QB])
                    xs_tiles[c] = xt

                # DMA issue order == transfer order (serial DMA pool in the
                # sim): prioritize the q path so the PE starts ~4.5us in,
                # then trickle kT/vT behind while Qproj chews.  The first
                # chunk and wq are split so the very first half-contraction
                # can begin after only two ~1.5us transfers.
                xt0 = xq_pool.tile([128, DC, QCH], bf16, tag="xq", name="xq0")
                xq_tiles[0] = xt0
                nc.sync.dma_start(out=xt0[:, 0:4, :], in_=qT_r[:, 0:4, 0:QCH])
                nc.sync.dma_start(out=wq_sb[:, 0:4, 0:256], in_=wq_r[:, 0:4, 0:256])
                nc.sync.dma_start(out=xt0[:, 4:8, :], in_=qT_r[:, 4:8, 0:QCH])
                nc.sync.dma_start(out=wq_sb[:, 4:8, 0:256], in_=wq_r[:, 4:8, 0:256])
                nc.sync.dma_start(out=xt0[:, 8:16, :], in_=qT_r[:, 8:16, 0:QCH])
                nc.sync.dma_start(out=wq_sb[:, 8:16, 0:256], in_=wq_r[:, 8:16, 0:256])
                nc.sync.dma_start(out=wq_sb[:, :, 256:512], in_=wq_r[:, :, 256:512])
                dma_q(1)
                dma_q(2)
                nc.sync.dma_start(out=wk_sb, in_=wk_r)
                dma_k(0)
                dma_q(3)
                dma_k(1)
                dma_q(4)
                dma_q(5)
                dma_k(2)
                dma_q(6)
                dma_k(3)
                nc.sync.dma_start(out=wv_sb, in_=wv_r)
                dma_v(0)
                dma_v(1)
                dma_v(2)
                dma_v(3)
                dma_q(7)
                # wo is not needed until the first out-proj group (~t+120us);
                # issuing it last keeps vT ahead of the V projection.
                nc.sync.dma_start(out=wo_sb, in_=wo_r)

                def qproj_head(c, h, psum_pool, tag):
                    xt = xq_tiles[c]
                    ps = psum_pool.tile([128, QB], f32, tag=tag)
                    for dc in range(DC):
                        nc.tensor.matmul(
                            ps[:, 0:QCH],
                            lhsT=wq_sb[:, dc, h * DH:(h + 1) * DH],
                            rhs=xt[:, dc, :],
                            start=(dc == 0),
                            stop=(dc == DC - 1),
                        )
                    nc.vector.tensor_copy(
                        qp[:, h, c * QCH:(c + 1) * QCH], ps[:, 0:QCH])

                def qproj(c):
                    for h in range(NH):
                        qproj_head(c, h, pj_psum, "pj")

                def kproj(c):
                    xt = xs_tiles[c]
                    ps = pj_psum.tile([128, QB], f32, tag="pj")
                    for dc in range(DC):
                        nc.tensor.matmul(
                            ps, lhsT=wk_sb[:, dc, :], rhs=xt[:, dc, :],
                            start=(dc == 0), stop=(dc == DC - 1),
                        )
                    nc.vector.tensor_copy(kp[:, c * QB:(c + 1) * QB], ps)

                # PE emission order tuned against DMA arrival times.
                # V2/V3 and Qproj(7) are deferred into the first attention
                # block so the PE has work while the tail of the stream
                # arrives.
                qproj(0)
                qproj(1)
                qproj(2)
                kproj(0)
                qproj(3)
                qproj(4)
                kproj(1)
                qproj(5)
                qproj(6)
                kproj(2)
                kproj(3)
                for kl in range(4):
                    vproj_unit(0, kl, pv_psum, "pv")
                for kl in range(4):
                    vproj_unit(1, kl, pv_psum, "pv")

            # ---- attention + interleaved output projection ----
            with tc.tile_pool(name="s_psum", bufs=2, space="PSUM") as s_psum, \
                 tc.tile_pool(name="av_psum", bufs=2, space="PSUM") as av_psum, \
                 tc.tile_pool(name="po_psum", bufs=2, space="PSUM") as po_psum, \
                 tc.tile_pool(name="pt_pool", bufs=6) as pt_pool, \
                 tc.tile_pool(name="small", bufs=3) as small_pool, \
                 tc.tile_pool(name="ostage", bufs=3) as ostage:

                def o_groups(qb):
                    """Generator: emit output projection for q rows of block
                    qb in 16 resumable chunks.  Each [sc, db] psum group is
                    split: ck0-2 accumulate immediately, ck3 (which reads the
                    h3-gated avn slice) is deferred one chunk so the PE has
                    runnable matmuls while the last head's normalization
                    lands."""
                    def part1(sc, db, n):
                        if qb == NQB - 1 and n % 2 == 1:
                            # drain phase: the scores ring is idle — borrow
                            # its banks to double the pipeline depth
                            st = s_psum.tile([128, 2, QB], f32, tag="s",
                                             name=f"pos{sc}_{db}")
                            po = st[:, 0, :]
                        else:
                            po = po_psum.tile([128, 512], f32, tag="po",
                                              name=f"po{sc}_{db}")
                        for ck in range(NH - 1):
                            nc.tensor.matmul(
                                po,
                                lhsT=avn[:, ck, sc * 128:(sc + 1) * 128],
                                rhs=wo_sb[:, ck, db * 512:(db + 1) * 512],
                                start=(ck == 0), stop=False,
                            )
                        return po

                    def finish(po, ot, sc, db, n):
                        nc.tensor.matmul(
                            po,
                            lhsT=avn[:, NH - 1, sc * 128:(sc + 1) * 128],
                            rhs=wo_sb[:, NH - 1, db * 512:(db + 1) * 512],
                            start=False, stop=True,
                        )
                        dst = ot[:, db * 512:(db + 1) * 512]
                        if n % 2 == 1:
                            nc.scalar.copy(dst, po)
                        else:
                            nc.vector.tensor_copy(dst, po)
                        nc.sync.dma_start(
                            out=out_r[:, sc, db * 512:(db + 1) * 512],
                            in_=dst)

                    prev = None
                    n = 0
                    for sc in range(qb * NQB, (qb + 1) * NQB):
                        ot = ostage.tile([128, D], bf16, tag="ot",
                                         name=f"ot{sc}")
                        for db in range(NH):
                            po = part1(sc, db, n)
                            if prev is not None:
                                finish(*prev, n)
                            n += 1
                            prev = (po, ot, sc, db)
                            yield
                    finish(*prev, n)

                def v_units():
                    for c in (2, 3):
                        for kl in range(4):
                            vproj_unit(c, kl, po_psum, "po")
                            yield

                # deferred Qproj(7) head-groups fill the ACT-paced idle of
                # the first attention block's later head iterations
                q7_fills = {
                    (0, 1, 1): 0, (0, 1, 5): 1, (0, 2, 1): 2, (0, 3, 1): 3,
                }

                pending_o = None
                pending_v = v_units()
                for qb in range(NQB):
                    qs = slice(qb * QB, (qb + 1) * QB)
                    for h in range(NH):
                        av = av_psum.tile([128, QB], f32, tag="av")
                        ptsum = small_pool.tile([128, QB], f16, tag="ptsum")
                        for pair in range(KC // 2):
                            ss = s_psum.tile([128, 2, QB], f32, tag="s")
                            for j in range(2):
                                kc = pair * 2 + j
                                nc.tensor.matmul(
                                    ss[:, j, :],
                                    lhsT=kp[:, kc * 128:(kc + 1) * 128],
                                    rhs=qp[:, h, qs],
                                    start=True, stop=True,
                                )
                            pt = pt_pool.tile([128, 2, QB], f16, tag="pt")
                            nc.scalar.activation(pt, ss, Exp, scale=SCALE)
                            # V2/V3 projection units ride in the first
                            # attention iteration's PE stream, as late as
                            # legality allows (units for vp[8+k] just before
                            # the AV of pair 4+k//2) so the vT-independent
                            # early pairs cover the stream's arrival.
                            if pending_v is not None and qb == 0 and h == 0 \
                                    and pair >= 4:
                                next(pending_v, None)
                                next(pending_v, None)
                            for j in range(2):
                                kc = pair * 2 + j
                                nc.tensor.matmul(
                                    av, lhsT=vp[:, kc, :], rhs=pt[:, j, :],
                                    start=(kc == 0), stop=(kc == KC - 1),
                                )
                            if pair == 0:
                                nc.vector.tensor_add(ptsum, pt[:, 0, :], pt[:, 1, :])
                            else:
                                nc.vector.tensor_add(ptsum, ptsum, pt[:, 0, :])
                                nc.vector.tensor_add(ptsum, ptsum, pt[:, 1, :])
                            # interleave one out-proj group into the PE stream
                            if pending_o is not None and pair % 2 == 1:
                                next(pending_o, None)
                            q7h = q7_fills.get((qb, h, pair))
                            if q7h is not None:
                                qproj_head(NQB * 2 - 1, q7h, po_psum, "po")
                        # softmax denominator: partition all-reduce (result
                        # replicated across partitions) on the idle gpsimd,
                        # then normalize via reciprocal + multiply on DVE.
                        rsum = small_pool.tile([128, QB], f32, tag="rsum")
                        nc.gpsimd.partition_all_reduce(
                            rsum, ptsum, channels=128,
                            reduce_op=bass_isa.ReduceOp.add)
                        rinv = small_pool.tile([128, QB], f32, tag="rinv")
                        if qb == NQB - 1 and h == NH - 1:
                            # the drain's deferred ck3 matmuls consume avn in
                            # 128-col slices; normalize in quarters so the
                            # first out-proj groups unblock ~1us earlier
                            for i in range(4):
                                sl = slice(i * 128, (i + 1) * 128)
                                nc.vector.reciprocal(rinv[:, sl], rsum[:, sl])
                                nc.vector.tensor_mul(
                                    avn[:, h, qb * QB + i * 128:
                                        qb * QB + (i + 1) * 128],
                                    av[:, sl], rinv[:, sl])
                        else:
                            nc.vector.reciprocal(rinv, rsum)
                            nc.vector.tensor_mul(avn[:, h, qs], av, rinv)
                    # drain leftover groups of the previous block, then arm
                    # this block's out-projection for interleaving
                    if pending_o is not None:
                        for _ in pending_o:
                            pass
                    pending_o = o_groups(qb)
                for _ in pending_o:
                    pass


def build_program():
    global _PROGRAM
    if _PROGRAM is not None:
        return _PROGRAM
    import concourse.tile as tile
    from concourse import bacc, bass_isa, mybir

    f32 = mybir.dt.float32
    bf16 = mybir.dt.bfloat16
    f16 = mybir.dt.float16
    nc = bacc.Bacc("TRN2", target_bir_lowering=False, debug=False)
    qT = nc.declare_dram_parameter("qT", [D, S], bf16, isOutput=False)
    kT = nc.declare_dram_parameter("kT", [D, S], bf16, isOutput=False)
    vT = nc.declare_dram_parameter("vT", [D, S], bf16, isOutput=False)
    wq = nc.declare_dram_parameter("wq", [D, NH * DH], bf16, isOutput=False)
    # wk/wv pre-rearranged on host to [128, DC*DH] (partition-major)
    wk = nc.declare_dram_parameter("wk", [128, DC * DH], bf16, isOutput=False)
    wv = nc.declare_dram_parameter("wv", [128, DC * DH], f16, isOutput=False)
    wo = nc.declare_dram_parameter("wo", [NH * DH, D], bf16, isOutput=False)
    out = nc.declare_dram_parameter("out", [S, D], bf16, isOutput=True)

    with tile.TileContext(nc) as tc:
        _emit(tc, nc, mybir, bass_isa, qT, kT, vT, wq, wk, wv, wo, out)

    nc.finalize()
    _PROGRAM = nc
    return nc


def _pmajor(w):
    # [D, DH] -> [128, DC*DH]: row (dc*128+p) becomes partition p, block dc
    return np.ascontiguousarray(
        w.reshape(DC, 128, DH).transpose(1, 0, 2).reshape(128, DC * DH))


def make_in_maps(query, key, value, Wq, Wk, Wv, Wo):
    bff = ml_dtypes.bfloat16
    in_maps = []
    for core in range(N_CORES):
        b, g = core // 4, core % 4
        in_maps.append({
            "qT": np.ascontiguousarray(np.asarray(query[b], np.float32).T).astype(bff),
            "kT": np.ascontiguousarray(np.asarray(key[b], np.float32).T).astype(bff),
            "vT": np.ascontiguousarray(np.asarray(value[b], np.float32).T).astype(bff),
            "wq": np.asarray(Wq[:, g * 512:(g + 1) * 512], np.float32).astype(bff),
            "wk": _pmajor(np.asarray(Wk[:, g * 128:(g + 1) * 128], np.float32)).astype(bff),
            "wv": _pmajor(np.asarray(Wv[:, g * 128:(g + 1) * 128], np.float32)).astype(np.float16),
            "wo": np.asarray(Wo[g * 512:(g + 1) * 512, :], np.float32).astype(bff),
        })
    return in_maps


def kernel(query, key, value, mask, Wq, Wk, Wv, Wo):
    global LAST_EXEC_NS, LAST_RESULTS
    del mask  # all-ones in this problem; softmax masking is a no-op
    nc = build_program()
    in_maps = make_in_maps(query, key, value, Wq, Wk, Wv, Wo)

    from concourse.bass_utils import run_bass_kernel_spmd

    res = run_bass_kernel_spmd(nc, in_maps, core_ids=list(range(N_CORES)))
    LAST_EXEC_NS = res.exec_time_ns
    LAST_RESULTS = res
    outs = [np.asarray(r["out"], dtype=np.float32) for r in res.results]
    full = np.empty((2, S, D), np.float32)
    for b in range(2):
        full[b] = outs[b * 4] + outs[b * 4 + 1] + outs[b * 4 + 2] + outs[b * 4 + 3]
    return full
